# revision 9
# baseline (speedup 1.0000x reference)
"""Sharded brute-force MIPS retrieval (top-k) on 8 Trainium2 NeuronCores.

Strategy (standard sharded FAISS pattern per the sharding hint):
  - passage_bank is sharded row-wise across 8 cores (62500 rows each),
    pre-transposed on the host to bankT [512, N] so both matmul operands
    have the embedding (contraction) dim on SBUF partitions.
  - Queries are replicated. Each core runs a bf16 screening matmul
    (fp32 is 4 cyc/col on the TRN2 PE, bf16 is 1 cyc/col and halves HBM
    traffic), producing fp32 scores in PSUM.
  - Top-k screening on device: scores are processed in 8192-wide windows.
    A segmented reduce_max (DVE, one pass) collapses each window to 512
    segment maxima (segments of 16 passages); Max8 + MaxIndex over the
    512 segment maxima yield the window's top-8 (value, segment) pairs.
    This costs ~1.1 DVE passes over the scores instead of 2 (max8 +
    max_index over raw scores), which is the kernel's bottleneck engine.
  - Host: merges 8 cores x 8 windows x 8 candidates per query, screens
    the top few dozen by value, enumerates each screened candidate's
    16-passage segment, rescores those exactly in float64 against the
    fp32 bank, and takes the global top-k. Because every screened
    candidate's whole segment is enumerated and rescored exactly, the
    final indices/scores are exact even though the screen ran in bf16.
"""

import numpy as np
import ml_dtypes
from contextlib import ExitStack

import concourse.bass as bass
import concourse.tile as tile
import concourse.mybir as mybir
from concourse import bacc
from concourse.bass_utils import run_bass_kernel_spmd

# Problem shapes (hardcoded per the harness contract).
D = 512          # embedding dim
B = 256          # batch (queries)
N_PASSAGES = 500000
N_CORES = 8
N_LOC = N_PASSAGES // N_CORES   # 62500 rows per core
W = 8192                        # score window
NW = 8                          # windows per core
N_PAD = W * NW                  # 65536 padded rows per core
SEG = 16                        # passages per segment
G_SEG = W // SEG                # 512 segments per window
K_CHUNKS = D // 128             # 4 contraction chunks
GROUPS = W // 512               # 16 matmul groups (one PSUM bank each)
SUB = 2048                      # bank sub-tile width (1 MiB DMAs)
CAND = NW * 8                   # 64 candidates per (core, query)

_program_cache = {}


def _build_program(repeat=1):
    if repeat in _program_cache:
        return _program_cache[repeat]

    nc = bacc.Bacc("TRN2", target_bir_lowering=False, debug=False)
    qT_d = nc.dram_tensor("qT", [D, B], mybir.dt.bfloat16, kind="ExternalInput").ap()
    bankT_d = nc.dram_tensor(
        "bankT", [D, N_PAD], mybir.dt.bfloat16, kind="ExternalInput"
    ).ap()
    vals_d = nc.dram_tensor("vals", [B, CAND], mybir.dt.float32, kind="ExternalOutput").ap()
    idx_d = nc.dram_tensor("idx", [B, CAND], mybir.dt.uint32, kind="ExternalOutput").ap()

    qT_v = qT_d.rearrange("(c p) b -> c p b", p=128)
    bankT_v = bankT_d.rearrange("(c p) n -> c p n", p=128)

    with tile.TileContext(nc) as tc:
        with ExitStack() as ctx:
            qpool = ctx.enter_context(tc.tile_pool(name="q", bufs=1))
            bpool = ctx.enter_context(tc.tile_pool(name="b", bufs=6))
            spool = ctx.enter_context(tc.tile_pool(name="s", bufs=1))
            gpool = ctx.enter_context(tc.tile_pool(name="g", bufs=2))
            pspool = ctx.enter_context(tc.tile_pool(name="p", bufs=1, space="PSUM"))
            apool = ctx.enter_context(tc.tile_pool(name="a", bufs=1))

            qt = qpool.tile([128, K_CHUNKS, B], mybir.dt.bfloat16, tag="qT")
            nc.sync.dma_start(qt[:], qT_v.rearrange("c p b -> p c b"))

            acc_v = [
                apool.tile([128, CAND], mybir.dt.float32, tag=f"av{i}", name=f"acc_v{i}")
                for i in range(2)
            ]
            acc_i = [
                apool.tile([128, CAND], mybir.dt.uint32, tag=f"ai{i}", name=f"acc_i{i}")
                for i in range(2)
            ]

            n_sub = W // SUB  # 4 bank sub-tiles per window
            for w in [w for _ in range(repeat) for w in range(NW)]:
                bsub = []
                for s in range(n_sub):
                    bt = bpool.tile([128, K_CHUNKS, SUB], mybir.dt.bfloat16,
                                    tag="bank", name=f"bank_w{w}_{s}")
                    lo = w * W + s * SUB
                    nc.sync.dma_start(
                        bt[:], bankT_v[:, :, lo:lo + SUB].rearrange("c p n -> p c n")
                    )
                    bsub.append(bt)
                for qi in range(2):
                    scores = spool.tile([128, W], mybir.dt.float32, tag=f"sc{qi}",
                                        name=f"scores_w{w}_q{qi}")
                    for half in range(2):
                        ps = {}
                        for e in range(K_CHUNKS):
                            for g8 in range(GROUPS // 2):
                                g = half * (GROUPS // 2) + g8
                                if e == 0:
                                    ps[g8] = pspool.tile([128, 512], mybir.dt.float32,
                                                         tag=f"ps{g8}", name=f"ps{g8}")
                                nc.tensor.matmul(
                                    ps[g8][:],
                                    qt[:, e, qi * 128:(qi + 1) * 128],
                                    bsub[g // 4][:, e, (g % 4) * 512:(g % 4 + 1) * 512],
                                    start=(e == 0),
                                    stop=(e == K_CHUNKS - 1),
                                )
                        for g8 in range(GROUPS // 2):
                            g = half * (GROUPS // 2) + g8
                            nc.scalar.copy(scores[:, g * 512:(g + 1) * 512], ps[g8][:])
                    seg = gpool.tile([128, G_SEG], mybir.dt.float32, tag=f"seg{qi}",
                                     name=f"seg_w{w}_q{qi}")
                    nc.vector.reduce_max(
                        seg[:], scores[:].rearrange("p (g s) -> p g s", s=SEG),
                        axis=mybir.AxisListType.X,
                    )
                    nc.vector.max(out=acc_v[qi][:, w * 8:(w + 1) * 8], in_=seg[:])
                    nc.vector.max_index(
                        out=acc_i[qi][:, w * 8:(w + 1) * 8],
                        in_max=acc_v[qi][:, w * 8:(w + 1) * 8],
                        in_values=seg[:],
                    )

            for qi in range(2):
                nc.sync.dma_start(vals_d[qi * 128:(qi + 1) * 128, :], acc_v[qi][:])
                nc.sync.dma_start(idx_d[qi * 128:(qi + 1) * 128, :], acc_i[qi][:])

    nc.compile()
    _program_cache[repeat] = nc
    return nc


def _prep_inputs(query_embeds, passage_bank):
    q = np.asarray(query_embeds, dtype=np.float32)
    bank = np.asarray(passage_bank, dtype=np.float32)

    q64 = q.astype(np.float64)
    norm = np.sqrt((q64 * q64).sum(axis=1, keepdims=True))
    q64 = q64 / np.maximum(norm, 1e-12)

    qT_bf = np.ascontiguousarray(q64.T.astype(np.float32)).astype(ml_dtypes.bfloat16)

    bank_bf = bank.astype(ml_dtypes.bfloat16)
    in_maps = []
    for c in range(N_CORES):
        shard = bank_bf[c * N_LOC:(c + 1) * N_LOC]       # [62500, 512]
        bt = np.zeros((D, N_PAD), dtype=ml_dtypes.bfloat16)
        bt[:, :N_LOC] = shard.T
        in_maps.append({"qT": qT_bf, "bankT": bt})
    return q64, in_maps


def _merge(results, q64, bank, passage_tokens, top_k, n_screen=24):
    # [B, N_CORES*CAND] candidate (value, padded-local-base) pairs
    all_vals = np.concatenate([r["vals"] for r in results], axis=1)
    win = np.arange(NW, dtype=np.int64).repeat(8)        # window id per slot
    seg_base = np.concatenate(
        [r["idx"].astype(np.int64) * SEG + win * W for r in results], axis=1
    )                                                    # padded-local segment start
    core_of = np.arange(N_CORES, dtype=np.int64).repeat(CAND)[None, :]

    valid = seg_base < N_LOC                             # zero-padding never ranks, but be safe
    screened = np.where(valid, all_vals, -np.inf)
    part = np.argpartition(-screened, n_screen, axis=1)[:, :n_screen]
    base = np.take_along_axis(seg_base, part, axis=1)            # [B, C]
    core = np.take_along_axis(np.broadcast_to(core_of, all_vals.shape), part, axis=1)

    # enumerate each screened candidate's segment (16 passages)
    local = base[:, :, None] + np.arange(SEG, dtype=np.int64)[None, None, :]
    ok = local < N_LOC
    gids = core[:, :, None] * N_LOC + np.minimum(local, N_LOC - 1)   # [B, C, SEG]
    Bsz, C = base.shape
    flat = gids.reshape(Bsz, C * SEG)
    ok = ok.reshape(Bsz, C * SEG)

    gathered = bank[flat.ravel()].astype(np.float64).reshape(Bsz, C * SEG, D)
    exact = np.einsum("bnd,bd->bn", gathered, q64)
    exact[~ok] = -np.inf
    # guard: if a segment were ever reported twice (fp32 value tie), keep one copy
    srt = np.argsort(flat, axis=1, kind="stable")
    sflat = np.take_along_axis(flat, srt, axis=1)
    dup_sorted = np.zeros_like(ok)
    dup_sorted[:, 1:] = sflat[:, 1:] == sflat[:, :-1]
    dup = np.zeros_like(ok)
    np.put_along_axis(dup, srt, dup_sorted, axis=1)
    exact[dup] = -np.inf
    order = np.lexsort((flat, -exact), axis=1)[:, :top_k]
    top_ids = np.take_along_axis(flat, order, axis=1)
    top_scores = np.take_along_axis(exact, order, axis=1).astype(np.float32)

    tokens = np.asarray(passage_tokens)
    retrieved = tokens[top_ids].reshape(Bsz, top_k * tokens.shape[1])
    return retrieved, top_scores


def kernel(query_embeds, passage_bank, passage_tokens, top_k):
    k = int(np.asarray(top_k))
    assert k <= 8, f"device kernel screens top-8 per window; top_k={k}"
    nc = _build_program()
    q64, in_maps = _prep_inputs(query_embeds, passage_bank)
    res = run_bass_kernel_spmd(nc, in_maps, core_ids=list(range(N_CORES)))
    bank = np.asarray(passage_bank, dtype=np.float32)
    return _merge(res.results, q64, bank, np.asarray(passage_tokens), k)


# revision 16
# speedup vs baseline: 1.2594x; 1.2594x over previous
"""Sharded brute-force MIPS retrieval (top-k) on 8 Trainium2 NeuronCores.

Strategy (standard sharded FAISS pattern per the sharding hint):
  - passage_bank is sharded row-wise across 8 cores (62500 rows each),
    pre-transposed on the host to bankT [512, N] so both matmul operands
    have the embedding (contraction) dim on SBUF partitions.
  - Queries are replicated. Each core runs a bf16 screening matmul
    (fp32 is 4 cyc/col on the TRN2 PE, bf16 is 1 cyc/col and halves HBM
    traffic), producing fp32 scores in PSUM.
  - Top-k screening on device: scores are processed in 8192-wide windows.
    A segmented reduce_max (DVE, one pass) collapses each window to 512
    segment maxima (segments of 16 passages); Max8 + MaxIndex over the
    512 segment maxima yield the window's top-8 (value, segment) pairs.
    This costs ~1.1 DVE passes over the scores instead of 2 (max8 +
    max_index over raw scores), which is the kernel's bottleneck engine.
  - Host: merges 8 cores x 8 windows x 8 candidates per query, screens
    the top few dozen by value, enumerates each screened candidate's
    16-passage segment, rescores those exactly in float64 against the
    fp32 bank, and takes the global top-k. Because every screened
    candidate's whole segment is enumerated and rescored exactly, the
    final indices/scores are exact even though the screen ran in bf16.
"""

import numpy as np
import ml_dtypes
from contextlib import ExitStack

import concourse.bass as bass
import concourse.tile as tile
import concourse.mybir as mybir
from concourse import bacc
from concourse.bass_utils import run_bass_kernel_spmd

# Problem shapes (hardcoded per the harness contract).
D = 512          # embedding dim
B = 256          # batch (queries)
N_PASSAGES = 500000
N_CORES = 8
N_LOC = N_PASSAGES // N_CORES   # 62500 rows per core
W = 8192                        # score window
NW = 8                          # windows per core
N_PAD = W * NW                  # 65536 padded rows per core
SEG = 16                        # passages per segment
G_SEG = W // SEG                # 512 segments per window
K_CHUNKS = D // 128             # 4 contraction chunks of 128
GROUPS = W // 512               # 16 matmul groups (one PSUM bank each)
SUB = 2048                      # bank sub-tile width
CAND = NW * 8                   # 64 candidates per (core, query)
USE_FP8 = True                  # fp8e4m3 + DoubleRow screen (2x PE, half DMA)
FP8_SCALE = 32.0                # pre-scale so ~N(0,1/512) elems are e4m3 normals

_program_cache = {}


def _build_program(repeat=1):
    if repeat in _program_cache:
        return _program_cache[repeat]

    nc = bacc.Bacc("TRN2", target_bir_lowering=False, debug=False)
    dt_in = mybir.dt.float8e4 if USE_FP8 else mybir.dt.bfloat16
    qT_d = nc.dram_tensor("qT", [D, B], dt_in, kind="ExternalInput").ap()
    bankT_d = nc.dram_tensor(
        "bankT", [D, N_PAD], dt_in, kind="ExternalInput"
    ).ap()
    vals_d = nc.dram_tensor("vals", [B, CAND], mybir.dt.float32, kind="ExternalOutput").ap()
    idx_d = nc.dram_tensor("idx", [B, CAND], mybir.dt.uint32, kind="ExternalOutput").ap()

    qT_v = qT_d.rearrange("(c p) b -> c p b", p=128)
    bankT_v = bankT_d.rearrange("(c p) n -> c p n", p=128)

    with tile.TileContext(nc) as tc:
        with ExitStack() as ctx:
            qpool = ctx.enter_context(tc.tile_pool(name="q", bufs=1))
            bpool = ctx.enter_context(tc.tile_pool(name="b", bufs=6))
            spool = ctx.enter_context(tc.tile_pool(name="s", bufs=1))
            gpool = ctx.enter_context(tc.tile_pool(name="g", bufs=2))
            pspool = ctx.enter_context(tc.tile_pool(name="p", bufs=1, space="PSUM"))
            apool = ctx.enter_context(tc.tile_pool(name="a", bufs=1))

            qt = qpool.tile([128, K_CHUNKS, B], dt_in, tag="qT")
            nc.sync.dma_start(qt[:], qT_v.rearrange("c p b -> p c b"))

            acc_v = [
                apool.tile([128, CAND], mybir.dt.float32, tag=f"av{i}", name=f"acc_v{i}")
                for i in range(2)
            ]
            acc_i = [
                apool.tile([128, CAND], mybir.dt.uint32, tag=f"ai{i}", name=f"acc_i{i}")
                for i in range(2)
            ]

            n_sub = W // SUB  # 4 bank sub-tiles per window
            for w in [w for _ in range(repeat) for w in range(NW)]:
                bsub = []
                for s in range(n_sub):
                    bt = bpool.tile([128, K_CHUNKS, SUB], dt_in,
                                    tag="bank", name=f"bank_w{w}_{s}")
                    lo = w * W + s * SUB
                    nc.sync.dma_start(
                        bt[:], bankT_v[:, :, lo:lo + SUB].rearrange("c p n -> p c n")
                    )
                    bsub.append(bt)
                for qi in range(2):
                    scores = spool.tile([128, W], mybir.dt.float32, tag=f"sc{qi}",
                                        name=f"scores_w{w}_q{qi}")
                    # quarter-window PSUM tiles (4 banks each, 2 in flight)
                    for quar in range(4):
                        ps = pspool.tile([128, 2048], mybir.dt.float32,
                                         tag="psq", name=f"ps_w{w}_q{qi}_{quar}")
                        sub = quar  # 2048-wide quarter == one bank sub-tile
                        if USE_FP8:
                            for e2 in range(2):  # contraction: 2 DoubleRow chunks of 256
                                for g4 in range(4):
                                    nc.tensor.matmul(
                                        ps[:, g4 * 512:(g4 + 1) * 512],
                                        qt[:, 2 * e2:2 * e2 + 2, qi * 128:(qi + 1) * 128],
                                        bsub[sub][:, 2 * e2:2 * e2 + 2,
                                                  g4 * 512:(g4 + 1) * 512],
                                        start=(e2 == 0),
                                        stop=(e2 == 1),
                                        perf_mode=mybir.MatmulPerfMode.DoubleRow,
                                    )
                        else:
                            for e in range(K_CHUNKS):
                                for g4 in range(4):
                                    nc.tensor.matmul(
                                        ps[:, g4 * 512:(g4 + 1) * 512],
                                        qt[:, e, qi * 128:(qi + 1) * 128],
                                        bsub[sub][:, e, g4 * 512:(g4 + 1) * 512],
                                        start=(e == 0),
                                        stop=(e == K_CHUNKS - 1),
                                    )
                        nc.scalar.copy(scores[:, quar * 2048:(quar + 1) * 2048], ps[:])
                    seg = gpool.tile([128, G_SEG], mybir.dt.float32, tag=f"seg{qi}",
                                     name=f"seg_w{w}_q{qi}")
                    nc.vector.reduce_max(
                        seg[:], scores[:].rearrange("p (g s) -> p g s", s=SEG),
                        axis=mybir.AxisListType.X,
                    )
                    nc.vector.max(out=acc_v[qi][:, w * 8:(w + 1) * 8], in_=seg[:])
                    nc.vector.max_index(
                        out=acc_i[qi][:, w * 8:(w + 1) * 8],
                        in_max=acc_v[qi][:, w * 8:(w + 1) * 8],
                        in_values=seg[:],
                    )

            for qi in range(2):
                nc.sync.dma_start(vals_d[qi * 128:(qi + 1) * 128, :], acc_v[qi][:])
                nc.sync.dma_start(idx_d[qi * 128:(qi + 1) * 128, :], acc_i[qi][:])

    nc.compile()
    _program_cache[repeat] = nc
    return nc


def _prep_inputs(query_embeds, passage_bank):
    q = np.asarray(query_embeds, dtype=np.float32)
    bank = np.asarray(passage_bank, dtype=np.float32)

    q64 = q.astype(np.float64)
    norm = np.sqrt((q64 * q64).sum(axis=1, keepdims=True))
    q64 = q64 / np.maximum(norm, 1e-12)

    if USE_FP8:
        dt_np = ml_dtypes.float8_e4m3
        scale = FP8_SCALE
    else:
        dt_np = ml_dtypes.bfloat16
        scale = 1.0

    qT_lo = np.ascontiguousarray((q64.T * scale).astype(np.float32)).astype(dt_np)

    bank_lo = (bank * np.float32(scale)).astype(dt_np)
    in_maps = []
    for c in range(N_CORES):
        shard = bank_lo[c * N_LOC:(c + 1) * N_LOC]       # [62500, 512]
        bt = np.zeros((D, N_PAD), dtype=dt_np)
        bt[:, :N_LOC] = shard.T
        in_maps.append({"qT": qT_lo, "bankT": bt})
    return q64, in_maps


def _merge(results, q64, bank, passage_tokens, top_k, n_screen=32):
    # [B, N_CORES*CAND] candidate (value, padded-local-base) pairs
    all_vals = np.concatenate([r["vals"] for r in results], axis=1)
    win = np.arange(NW, dtype=np.int64).repeat(8)        # window id per slot
    seg_base = np.concatenate(
        [r["idx"].astype(np.int64) * SEG + win * W for r in results], axis=1
    )                                                    # padded-local segment start
    core_of = np.arange(N_CORES, dtype=np.int64).repeat(CAND)[None, :]

    valid = seg_base < N_LOC                             # zero-padding never ranks, but be safe
    screened = np.where(valid, all_vals, -np.inf)
    part = np.argpartition(-screened, n_screen, axis=1)[:, :n_screen]
    base = np.take_along_axis(seg_base, part, axis=1)            # [B, C]
    core = np.take_along_axis(np.broadcast_to(core_of, all_vals.shape), part, axis=1)

    # enumerate each screened candidate's segment (16 passages)
    local = base[:, :, None] + np.arange(SEG, dtype=np.int64)[None, None, :]
    ok = local < N_LOC
    gids = core[:, :, None] * N_LOC + np.minimum(local, N_LOC - 1)   # [B, C, SEG]
    Bsz, C = base.shape
    flat = gids.reshape(Bsz, C * SEG)
    ok = ok.reshape(Bsz, C * SEG)

    gathered = bank[flat.ravel()].astype(np.float64).reshape(Bsz, C * SEG, D)
    exact = np.einsum("bnd,bd->bn", gathered, q64)
    exact[~ok] = -np.inf
    # guard: if a segment were ever reported twice (fp32 value tie), keep one copy
    srt = np.argsort(flat, axis=1, kind="stable")
    sflat = np.take_along_axis(flat, srt, axis=1)
    dup_sorted = np.zeros_like(ok)
    dup_sorted[:, 1:] = sflat[:, 1:] == sflat[:, :-1]
    dup = np.zeros_like(ok)
    np.put_along_axis(dup, srt, dup_sorted, axis=1)
    exact[dup] = -np.inf
    order = np.lexsort((flat, -exact), axis=1)[:, :top_k]
    top_ids = np.take_along_axis(flat, order, axis=1)
    top_scores = np.take_along_axis(exact, order, axis=1).astype(np.float32)

    tokens = np.asarray(passage_tokens)
    retrieved = tokens[top_ids].reshape(Bsz, top_k * tokens.shape[1])
    return retrieved, top_scores


def kernel(query_embeds, passage_bank, passage_tokens, top_k):
    k = int(np.asarray(top_k))
    assert k <= 8, f"device kernel screens top-8 per window; top_k={k}"
    nc = _build_program()
    q64, in_maps = _prep_inputs(query_embeds, passage_bank)
    res = run_bass_kernel_spmd(nc, in_maps, core_ids=list(range(N_CORES)))
    bank = np.asarray(passage_bank, dtype=np.float32)
    return _merge(res.results, q64, bank, np.asarray(passage_tokens), k)


# revision 20
# speedup vs baseline: 1.2963x; 1.0293x over previous
"""Sharded brute-force MIPS retrieval (top-k) on 8 Trainium2 NeuronCores.

Strategy (standard sharded FAISS pattern per the sharding hint):
  - passage_bank is sharded row-wise across 8 cores (62500 rows each),
    pre-transposed on the host to bankT [512, N] so both matmul operands
    have the embedding (contraction) dim on SBUF partitions.
  - Queries are replicated. Each core runs a bf16 screening matmul
    (fp32 is 4 cyc/col on the TRN2 PE, bf16 is 1 cyc/col and halves HBM
    traffic), producing fp32 scores in PSUM.
  - Top-k screening on device: scores are processed in 8192-wide windows.
    A segmented reduce_max (DVE, one pass) collapses each window to 512
    segment maxima (segments of 16 passages); Max8 + MaxIndex over the
    512 segment maxima yield the window's top-8 (value, segment) pairs.
    This costs ~1.1 DVE passes over the scores instead of 2 (max8 +
    max_index over raw scores), which is the kernel's bottleneck engine.
  - Host: merges 8 cores x 8 windows x 8 candidates per query, screens
    the top few dozen by value, enumerates each screened candidate's
    16-passage segment, rescores those exactly in float64 against the
    fp32 bank, and takes the global top-k. Because every screened
    candidate's whole segment is enumerated and rescored exactly, the
    final indices/scores are exact even though the screen ran in bf16.
"""

import numpy as np
import ml_dtypes
from contextlib import ExitStack

import concourse.bass as bass
import concourse.tile as tile
import concourse.mybir as mybir
from concourse import bacc
from concourse.bass_utils import run_bass_kernel_spmd

# Problem shapes (hardcoded per the harness contract).
D = 512          # embedding dim
B = 256          # batch (queries)
N_PASSAGES = 500000
N_CORES = 8
N_LOC = N_PASSAGES // N_CORES   # 62500 rows per core
W = 8192                        # score window
NW = 8                          # windows per core
N_PAD = W * NW                  # 65536 padded rows per core
SEG = 32                        # passages per segment
G_SEG = W // SEG                # 512 segments per window
K_CHUNKS = D // 128             # 4 contraction chunks of 128
GROUPS = W // 512               # 16 matmul groups (one PSUM bank each)
SUB = 2048                      # bank sub-tile width
CAND = NW * 8                   # 64 candidates per (core, query)
USE_FP8 = True                  # fp8e4m3 + DoubleRow screen (2x PE, half DMA)
FP8_SCALE = 32.0                # pre-scale so ~N(0,1/512) elems are e4m3 normals

_program_cache = {}


def _build_program(repeat=1):
    if repeat in _program_cache:
        return _program_cache[repeat]

    nc = bacc.Bacc("TRN2", target_bir_lowering=False, debug=False)
    dt_in = mybir.dt.float8e4 if USE_FP8 else mybir.dt.bfloat16
    qT_d = nc.dram_tensor("qT", [D, B], dt_in, kind="ExternalInput").ap()
    bankT_d = nc.dram_tensor(
        "bankT", [D, N_PAD], dt_in, kind="ExternalInput"
    ).ap()
    vals_d = nc.dram_tensor("vals", [B, CAND], mybir.dt.float32, kind="ExternalOutput").ap()
    idx_d = nc.dram_tensor("idx", [B, CAND], mybir.dt.uint32, kind="ExternalOutput").ap()

    qT_v = qT_d.rearrange("(c p) b -> c p b", p=128)
    bankT_v = bankT_d.rearrange("(c p) n -> c p n", p=128)

    with tile.TileContext(nc) as tc:
        with ExitStack() as ctx:
            qpool = ctx.enter_context(tc.tile_pool(name="q", bufs=1))
            bpool = ctx.enter_context(tc.tile_pool(name="b", bufs=6))
            spool = ctx.enter_context(tc.tile_pool(name="s", bufs=2))
            gpool = ctx.enter_context(tc.tile_pool(name="g", bufs=2))
            pspool = ctx.enter_context(tc.tile_pool(name="p", bufs=2, space="PSUM"))
            apool = ctx.enter_context(tc.tile_pool(name="a", bufs=1))

            qt = qpool.tile([128, K_CHUNKS, B], dt_in, tag="qT")
            nc.sync.dma_start(qt[:], qT_v.rearrange("c p b -> p c b"))

            acc_v = [
                apool.tile([128, CAND], mybir.dt.float32, tag=f"av{i}", name=f"acc_v{i}")
                for i in range(2)
            ]
            acc_i = [
                apool.tile([128, CAND], mybir.dt.uint32, tag=f"ai{i}", name=f"acc_i{i}")
                for i in range(2)
            ]

            n_sub = W // SUB  # 4 bank sub-tiles per window
            for w in [w for _ in range(repeat) for w in range(NW)]:
                bsub = []
                for s in range(n_sub):
                    bt = bpool.tile([128, K_CHUNKS, SUB], dt_in,
                                    tag="bank", name=f"bank_w{w}_{s}")
                    lo = w * W + s * SUB
                    nc.sync.dma_start(
                        bt[:], bankT_v[:, :, lo:lo + SUB].rearrange("c p n -> p c n")
                    )
                    bsub.append(bt)
                scores = [
                    spool.tile([128, W], mybir.dt.float32, tag=f"sc{qi}",
                               name=f"scores_w{w}_q{qi}")
                    for qi in range(2)
                ]
                # 2048-wide PSUM tiles (4 banks each, 2 in flight)
                for quar in range(W // 2048):
                    for qi in range(2):
                        ps = pspool.tile([128, 2048], mybir.dt.float32,
                                         tag="psq", name=f"ps_w{w}_q{qi}_{quar}")
                        sub = quar  # 2048-wide quarter == one bank sub-tile
                        if USE_FP8:
                            for e2 in range(2):  # contraction: 2 DoubleRow chunks of 256
                                for g4 in range(4):
                                    nc.tensor.matmul(
                                        ps[:, g4 * 512:(g4 + 1) * 512],
                                        qt[:, 2 * e2:2 * e2 + 2, qi * 128:(qi + 1) * 128],
                                        bsub[sub][:, 2 * e2:2 * e2 + 2,
                                                  g4 * 512:(g4 + 1) * 512],
                                        start=(e2 == 0),
                                        stop=(e2 == 1),
                                        perf_mode=mybir.MatmulPerfMode.DoubleRow,
                                    )
                        else:
                            for e in range(K_CHUNKS):
                                for g4 in range(4):
                                    nc.tensor.matmul(
                                        ps[:, g4 * 512:(g4 + 1) * 512],
                                        qt[:, e, qi * 128:(qi + 1) * 128],
                                        bsub[sub][:, e, g4 * 512:(g4 + 1) * 512],
                                        start=(e == 0),
                                        stop=(e == K_CHUNKS - 1),
                                    )
                        nc.scalar.copy(scores[qi][:, quar * 2048:(quar + 1) * 2048], ps[:])
                for qi in range(2):
                    seg = gpool.tile([128, G_SEG], mybir.dt.float32, tag=f"seg{qi}",
                                     name=f"seg_w{w}_q{qi}")
                    nc.vector.reduce_max(
                        seg[:], scores[qi][:].rearrange("p (g s) -> p g s", s=SEG),
                        axis=mybir.AxisListType.X,
                    )
                    nc.vector.max(out=acc_v[qi][:, w * 8:(w + 1) * 8], in_=seg[:])
                    nc.vector.max_index(
                        out=acc_i[qi][:, w * 8:(w + 1) * 8],
                        in_max=acc_v[qi][:, w * 8:(w + 1) * 8],
                        in_values=seg[:],
                    )

            for qi in range(2):
                nc.sync.dma_start(vals_d[qi * 128:(qi + 1) * 128, :], acc_v[qi][:])
                nc.sync.dma_start(idx_d[qi * 128:(qi + 1) * 128, :], acc_i[qi][:])

    nc.compile()
    _program_cache[repeat] = nc
    return nc


def _prep_inputs(query_embeds, passage_bank):
    q = np.asarray(query_embeds, dtype=np.float32)
    bank = np.asarray(passage_bank, dtype=np.float32)

    q64 = q.astype(np.float64)
    norm = np.sqrt((q64 * q64).sum(axis=1, keepdims=True))
    q64 = q64 / np.maximum(norm, 1e-12)

    if USE_FP8:
        dt_np = ml_dtypes.float8_e4m3
        scale = FP8_SCALE
    else:
        dt_np = ml_dtypes.bfloat16
        scale = 1.0

    qT_lo = np.ascontiguousarray((q64.T * scale).astype(np.float32)).astype(dt_np)

    bank_lo = (bank * np.float32(scale)).astype(dt_np)
    in_maps = []
    for c in range(N_CORES):
        shard = bank_lo[c * N_LOC:(c + 1) * N_LOC]       # [62500, 512]
        bt = np.zeros((D, N_PAD), dtype=dt_np)
        bt[:, :N_LOC] = shard.T
        in_maps.append({"qT": qT_lo, "bankT": bt})
    return q64, in_maps


def _merge(results, q64, bank, passage_tokens, top_k, n_screen=32):
    # [B, N_CORES*CAND] candidate (value, padded-local-base) pairs
    all_vals = np.concatenate([r["vals"] for r in results], axis=1)
    win = np.arange(NW, dtype=np.int64).repeat(8)        # window id per slot
    seg_base = np.concatenate(
        [r["idx"].astype(np.int64) * SEG + win * W for r in results], axis=1
    )                                                    # padded-local segment start
    core_of = np.arange(N_CORES, dtype=np.int64).repeat(CAND)[None, :]

    valid = seg_base < N_LOC                             # zero-padding never ranks, but be safe
    screened = np.where(valid, all_vals, -np.inf)
    part = np.argpartition(-screened, n_screen, axis=1)[:, :n_screen]
    base = np.take_along_axis(seg_base, part, axis=1)            # [B, C]
    core = np.take_along_axis(np.broadcast_to(core_of, all_vals.shape), part, axis=1)

    # enumerate each screened candidate's segment (16 passages)
    local = base[:, :, None] + np.arange(SEG, dtype=np.int64)[None, None, :]
    ok = local < N_LOC
    gids = core[:, :, None] * N_LOC + np.minimum(local, N_LOC - 1)   # [B, C, SEG]
    Bsz, C = base.shape
    flat = gids.reshape(Bsz, C * SEG)
    ok = ok.reshape(Bsz, C * SEG)

    gathered = bank[flat.ravel()].astype(np.float64).reshape(Bsz, C * SEG, D)
    exact = np.einsum("bnd,bd->bn", gathered, q64)
    exact[~ok] = -np.inf
    # guard: if a segment were ever reported twice (fp32 value tie), keep one copy
    srt = np.argsort(flat, axis=1, kind="stable")
    sflat = np.take_along_axis(flat, srt, axis=1)
    dup_sorted = np.zeros_like(ok)
    dup_sorted[:, 1:] = sflat[:, 1:] == sflat[:, :-1]
    dup = np.zeros_like(ok)
    np.put_along_axis(dup, srt, dup_sorted, axis=1)
    exact[dup] = -np.inf
    order = np.lexsort((flat, -exact), axis=1)[:, :top_k]
    top_ids = np.take_along_axis(flat, order, axis=1)
    top_scores = np.take_along_axis(exact, order, axis=1).astype(np.float32)

    tokens = np.asarray(passage_tokens)
    retrieved = tokens[top_ids].reshape(Bsz, top_k * tokens.shape[1])
    return retrieved, top_scores


def kernel(query_embeds, passage_bank, passage_tokens, top_k):
    k = int(np.asarray(top_k))
    assert k <= 8, f"device kernel screens top-8 per window; top_k={k}"
    nc = _build_program()
    q64, in_maps = _prep_inputs(query_embeds, passage_bank)
    res = run_bass_kernel_spmd(nc, in_maps, core_ids=list(range(N_CORES)))
    bank = np.asarray(passage_bank, dtype=np.float32)
    return _merge(res.results, q64, bank, np.asarray(passage_tokens), k)


# revision 22
# speedup vs baseline: 619.7927x; 478.1313x over previous
"""Sharded brute-force MIPS retrieval (top-k) on 8 Trainium2 NeuronCores.

Strategy (standard sharded FAISS pattern per the sharding hint):
  - passage_bank is sharded row-wise across 8 cores (62500 rows each),
    pre-transposed on the host to bankT [512, N] so both matmul operands
    have the embedding (contraction) dim on SBUF partitions.
  - Queries are replicated. Each core runs an fp8(e4m3)+DoubleRow
    screening matmul (fp32 is 4 cyc/col on the TRN2 PE; fp8 DoubleRow is
    ~2x bf16 rate and 1/4 the HBM traffic), accumulating fp32 scores in
    PSUM. Inputs are pre-scaled by 32 so N(0,1/512) elements land in the
    e4m3 normal range; scaling is rank-preserving.
  - Screening on device: scores stream through a segmented reduce_max
    (DVE, one pass) into per-core segment maxima (segments of 32
    passages, 1984 segments/core per query). One Max8 + MaxIndex over
    the segment maxima yields each core's top-8 (value, segment) pairs.
    This is ~1 DVE pass over the scores instead of the 2 a direct
    max8+max_index would need; the DVE is the bottleneck engine.
    Per-core top-8 suffices: at most 7 items globally exceed any true
    top-8 item, so its segment max always ranks in its core's top-8
    (the fp8 screen noise is far too small to add 7+ false competitors).
  - Host: merges 8 cores x 8 candidates per query, screens the top 32
    by value, enumerates each screened candidate's 32-passage segment,
    rescores those exactly in float64 against the fp32 bank, and takes
    the global top-k. Because every screened candidate's whole segment
    is enumerated and rescored exactly, the final indices/scores are
    exact even though the screen ran in fp8.
"""

import numpy as np
import ml_dtypes
from contextlib import ExitStack

import concourse.bass as bass
import concourse.tile as tile
import concourse.mybir as mybir
from concourse import bacc
from concourse.bass_utils import run_bass_kernel_spmd

# Problem shapes (hardcoded per the harness contract).
D = 512          # embedding dim
B = 256          # batch (queries)
N_PASSAGES = 500000
N_CORES = 8
N_LOC = N_PASSAGES // N_CORES   # 62500 rows per core
WS = [4096, 4096] + [8192] * 6 + [6144]  # windows (sum = 63488 padded rows)
N_PAD = sum(WS)
SEG = 32                        # passages per segment
G_ALL = N_PAD // SEG            # 1984 segments per core
K_CHUNKS = D // 128             # 4 contraction chunks of 128
SUB = 2048                      # bank sub-tile / PSUM tile width
CAND = 8                        # top-8 per core per query
USE_FP8 = True                  # fp8e4m3 + DoubleRow screen
FP8_SCALE = 32.0                # pre-scale so ~N(0,1/512) elems are e4m3 normals

_program_cache = {}


def _build_program(repeat=1):
    if repeat in _program_cache:
        return _program_cache[repeat]

    nc = bacc.Bacc("TRN2", target_bir_lowering=False, debug=False)
    dt_in = mybir.dt.float8e4 if USE_FP8 else mybir.dt.bfloat16
    qT_d = nc.dram_tensor("qT", [D, B], dt_in, kind="ExternalInput").ap()
    bankT_d = nc.dram_tensor(
        "bankT", [D, N_PAD], dt_in, kind="ExternalInput"
    ).ap()
    vals_d = nc.dram_tensor("vals", [B, CAND], mybir.dt.float32, kind="ExternalOutput").ap()
    idx_d = nc.dram_tensor("idx", [B, CAND], mybir.dt.uint32, kind="ExternalOutput").ap()

    qT_v = qT_d.rearrange("(c p) b -> c p b", p=128)
    bankT_v = bankT_d.rearrange("(c p) n -> c p n", p=128)

    with tile.TileContext(nc) as tc:
        with ExitStack() as ctx:
            qpool = ctx.enter_context(tc.tile_pool(name="q", bufs=1))
            bpool = ctx.enter_context(tc.tile_pool(name="b", bufs=6))
            spool = ctx.enter_context(tc.tile_pool(name="s", bufs=2))
            gpool = ctx.enter_context(tc.tile_pool(name="g", bufs=1))
            pspool = ctx.enter_context(tc.tile_pool(name="p", bufs=2, space="PSUM"))
            apool = ctx.enter_context(tc.tile_pool(name="a", bufs=1))

            qt = qpool.tile([128, K_CHUNKS, B], dt_in, tag="qT")
            nc.sync.dma_start(qt[:], qT_v.rearrange("c p b -> p c b"))

            seg_acc = [
                gpool.tile([128, G_ALL], mybir.dt.float32, tag=f"seg{i}", name=f"seg_acc{i}")
                for i in range(2)
            ]

            for rep in range(repeat):
                off = 0
                for w, Ww in enumerate(WS):
                    n_sub = Ww // SUB
                    bsub = []
                    for s in range(n_sub):
                        bt = bpool.tile([128, K_CHUNKS, SUB], dt_in,
                                        tag="bank", name=f"bank_w{w}_{s}")
                        lo = off + s * SUB
                        nc.sync.dma_start(
                            bt[:], bankT_v[:, :, lo:lo + SUB].rearrange("c p n -> p c n")
                        )
                        bsub.append(bt)
                    scores = [
                        spool.tile([128, Ww], mybir.dt.float32, tag=f"sc{qi}",
                                   name=f"scores_w{w}_q{qi}")
                        for qi in range(2)
                    ]
                    for quar in range(n_sub):
                        for qi in range(2):
                            ps = pspool.tile([128, SUB], mybir.dt.float32,
                                             tag="psq", name=f"ps_w{w}_q{qi}_{quar}")
                            if USE_FP8:
                                for e2 in range(2):  # 2 DoubleRow chunks of K=256
                                    for g4 in range(4):
                                        nc.tensor.matmul(
                                            ps[:, g4 * 512:(g4 + 1) * 512],
                                            qt[:, 2 * e2:2 * e2 + 2, qi * 128:(qi + 1) * 128],
                                            bsub[quar][:, 2 * e2:2 * e2 + 2,
                                                       g4 * 512:(g4 + 1) * 512],
                                            start=(e2 == 0),
                                            stop=(e2 == 1),
                                            perf_mode=mybir.MatmulPerfMode.DoubleRow,
                                        )
                            else:
                                for e in range(K_CHUNKS):
                                    for g4 in range(4):
                                        nc.tensor.matmul(
                                            ps[:, g4 * 512:(g4 + 1) * 512],
                                            qt[:, e, qi * 128:(qi + 1) * 128],
                                            bsub[quar][:, e, g4 * 512:(g4 + 1) * 512],
                                            start=(e == 0),
                                            stop=(e == K_CHUNKS - 1),
                                        )
                            nc.scalar.copy(scores[qi][:, quar * SUB:(quar + 1) * SUB], ps[:])
                    for qi in range(2):
                        nc.vector.reduce_max(
                            seg_acc[qi][:, off // SEG:(off + Ww) // SEG],
                            scores[qi][:].rearrange("p (g s) -> p g s", s=SEG),
                            axis=mybir.AxisListType.X,
                        )
                    off += Ww

                acc_v = [
                    apool.tile([128, CAND], mybir.dt.float32, tag=f"av{i}",
                               name=f"acc_v{i}_r{rep}")
                    for i in range(2)
                ]
                acc_i = [
                    apool.tile([128, CAND], mybir.dt.uint32, tag=f"ai{i}",
                               name=f"acc_i{i}_r{rep}")
                    for i in range(2)
                ]
                for qi in range(2):
                    nc.vector.max(out=acc_v[qi][:], in_=seg_acc[qi][:])
                    nc.vector.max_index(
                        out=acc_i[qi][:], in_max=acc_v[qi][:], in_values=seg_acc[qi][:]
                    )
                if rep == repeat - 1:
                    for qi in range(2):
                        nc.sync.dma_start(vals_d[qi * 128:(qi + 1) * 128, :], acc_v[qi][:])
                        nc.sync.dma_start(idx_d[qi * 128:(qi + 1) * 128, :], acc_i[qi][:])

    nc.compile()
    _program_cache[repeat] = nc
    return nc


def _prep_inputs(query_embeds, passage_bank):
    q = np.asarray(query_embeds, dtype=np.float32)
    bank = np.asarray(passage_bank, dtype=np.float32)

    q64 = q.astype(np.float64)
    norm = np.sqrt((q64 * q64).sum(axis=1, keepdims=True))
    q64 = q64 / np.maximum(norm, 1e-12)

    if USE_FP8:
        dt_np = ml_dtypes.float8_e4m3
        scale = FP8_SCALE
    else:
        dt_np = ml_dtypes.bfloat16
        scale = 1.0

    qT_lo = np.ascontiguousarray((q64.T * scale).astype(np.float32)).astype(dt_np)

    bank_lo = (bank * np.float32(scale)).astype(dt_np)
    in_maps = []
    for c in range(N_CORES):
        shard = bank_lo[c * N_LOC:(c + 1) * N_LOC]       # [62500, 512]
        bt = np.zeros((D, N_PAD), dtype=dt_np)
        bt[:, :N_LOC] = shard.T
        in_maps.append({"qT": qT_lo, "bankT": bt})
    return q64, in_maps


def _merge(results, q64, bank, passage_tokens, top_k, n_screen=32):
    # [B, N_CORES*CAND] candidate (value, padded-local segment base) pairs
    all_vals = np.concatenate([r["vals"] for r in results], axis=1)
    seg_base = np.concatenate(
        [r["idx"].astype(np.int64) * SEG for r in results], axis=1
    )
    core_of = np.arange(N_CORES, dtype=np.int64).repeat(CAND)[None, :]

    valid = seg_base < N_LOC                 # zero-padding never ranks, but be safe
    screened = np.where(valid, all_vals, -np.inf)
    part = np.argpartition(-screened, n_screen, axis=1)[:, :n_screen]
    base = np.take_along_axis(seg_base, part, axis=1)            # [B, C]
    core = np.take_along_axis(np.broadcast_to(core_of, all_vals.shape), part, axis=1)

    # enumerate each screened candidate's segment (SEG passages)
    local = base[:, :, None] + np.arange(SEG, dtype=np.int64)[None, None, :]
    ok = local < N_LOC
    gids = core[:, :, None] * N_LOC + np.minimum(local, N_LOC - 1)   # [B, C, SEG]
    Bsz, C = base.shape
    flat = gids.reshape(Bsz, C * SEG)
    ok = ok.reshape(Bsz, C * SEG)

    gathered = bank[flat.ravel()].astype(np.float64).reshape(Bsz, C * SEG, D)
    exact = np.einsum("bnd,bd->bn", gathered, q64)
    exact[~ok] = -np.inf
    # guard: if a segment were ever reported twice (fp32 value tie), keep one copy
    srt = np.argsort(flat, axis=1, kind="stable")
    sflat = np.take_along_axis(flat, srt, axis=1)
    dup_sorted = np.zeros_like(ok)
    dup_sorted[:, 1:] = sflat[:, 1:] == sflat[:, :-1]
    dup = np.zeros_like(ok)
    np.put_along_axis(dup, srt, dup_sorted, axis=1)
    exact[dup] = -np.inf
    order = np.lexsort((flat, -exact), axis=1)[:, :top_k]
    top_ids = np.take_along_axis(flat, order, axis=1)
    top_scores = np.take_along_axis(exact, order, axis=1).astype(np.float32)

    tokens = np.asarray(passage_tokens)
    retrieved = tokens[top_ids].reshape(Bsz, top_k * tokens.shape[1])
    return retrieved, top_scores


def kernel(query_embeds, passage_bank, passage_tokens, top_k):
    k = int(np.asarray(top_k))
    assert k <= 8, f"device kernel screens top-8 per core; top_k={k}"
    nc = _build_program()
    q64, in_maps = _prep_inputs(query_embeds, passage_bank)
    res = run_bass_kernel_spmd(nc, in_maps, core_ids=list(range(N_CORES)))
    bank = np.asarray(passage_bank, dtype=np.float32)
    return _merge(res.results, q64, bank, np.asarray(passage_tokens), k)
